# revision 1
# baseline (speedup 1.0000x reference)
"""Trainium2 Bass kernel for the FGWF objective:

    out = sum(cost_mat(graph, graph_b, prob, prob_b, tran, embedding, embedding_b) * tran)

Decomposition:
  sum(cost*T) = sum_i (f1_i + 0.5*||E_i||^2) * rowsum(T)_i
              + sum_j (f2_j + 0.5*||Eb_j||^2) * colsum(T)_j
              - 2 * <G @ T, T @ Gb>              (two 2048^3 matmuls, fp8 DoubleRow)
              - <E, T @ Eb>                      (feature cross term)
  with f1 = (G^2) @ p_s, f2 = (Gb^2) @ p_t.

Device computes only the O(N^3) terms: A = G@T and B = T@Gb tiles with a fused
<A,B> accumulation, plus the O(N^2 D) feature term (T Eb)^T dotted with E.  All
O(N^2) rank-1 corrections (f1, f2, rowsum/colsum of T, embedding norms) are
computed on the host in f64 during the reduction -- they are tiny and keeping
them off the PE shortens the matmul stream.

Sharding over 8 NeuronCores: 2D grid (4 row-blocks x 2 col-blocks).  Core
(r, c) computes A = G[rblk] @ T[:, chalf], B = T[rblk] @ Gb[:, chalf] and the
(rblk, dhalf) slice of the feature term.  tran is pre-scaled by 2^20 on the
host so fp8-e4m3 avoids subnormal flush; partials are rescaled on the host.
"""
import numpy as np
import ml_dtypes

import concourse.bass as bass
from concourse import mybir
from concourse import bass_utils
from concourse.tile import TileContext

FP8 = ml_dtypes.float8_e4m3
F32 = mybir.dt.float32
F8 = mybir.dt.float8e4
TSCALE = 2.0 ** 20
N = 2048
NCORES = 8
NWARM = 37

_cache = {}


def _split_waits(nc):
    """The walrus build here rejects >1 sem-wait per instruction; hoist extra
    waits onto preceding same-engine nops."""
    MAXW = 1
    for fn in nc.m.functions:
        for b in fn.blocks:
            out = []
            changed = False
            for inst in b.instructions:
                si = inst.sync_info
                waits = list(si.on_wait) if si and si.on_wait else []
                if len(waits) > MAXW:
                    changed = True
                    k = 0
                    while len(waits) > MAXW:
                        chunk, waits = waits[:MAXW], waits[MAXW:]
                        out.append(mybir.InstNoOp(
                            name=f"{inst.name}-wsplit{k}", engine=inst.engine,
                            sync_info=mybir.SyncInfo(on_wait=chunk, on_update=[]),
                            bass_nofuse=True))
                        k += 1
                    inst.sync_info = mybir.SyncInfo(
                        on_wait=waits,
                        on_update=list(si.on_update) if si.on_update else [])
                out.append(inst)
            if changed:
                b.instructions = out

def _build():
    nc = bass.Bass()
    AL = mybir.AluOpType
    DR = mybir.MatmulPerfMode.DoubleRow

    # per-core inputs (host-sharded).  ap packs [G[rblk]^T | T[:,c0]] and bp
    # packs [T[rblk]^T | Gb[:,c0] | Gb[:,c1] | Eb[:,dh]] row-wise so each DMA
    # piece carries long contiguous per-partition runs (big packets -> full
    # HBM bandwidth) while still landing chunk-pair granular for pacing.
    ap = nc.declare_dram_parameter("ap", [N, 1024], F8, isOutput=False)
    tc1 = nc.declare_dram_parameter("tc1", [N, 512], F8, isOutput=False)
    bp = nc.declare_dram_parameter("bp", [N, 1600], F8, isOutput=False)
    efT = nc.declare_dram_parameter("efT", [64, 512], F32, isOutput=False)

    out_d = nc.declare_dram_parameter("out", [128, 9], F32, isOutput=True)

    with TileContext(nc) as tc:
        with (
            tc.tile_pool(name="big", bufs=1) as big,
            tc.tile_pool(name="asb", bufs=8) as asb_p,
            tc.tile_pool(name="tout", bufs=2) as tout_p,
            tc.tile_pool(name="pmain", bufs=7, space="PSUM") as pmain_p,
            tc.tile_pool(name="pwe", bufs=1, space="PSUM") as pwe_p,
        ):
            ap_sb = big.tile([128, 16, 1024], F8, tag="ap")
            tc1_sb = big.tile([128, 16, 512], F8, tag="tc1")
            bp_sb = big.tile([128, 16, 1600], F8, tag="bp")
            efT_sb = big.tile([64, 512], F32, tag="efT")
            csT_sb = ap_sb[:, :, 0:512]
            tc0_sb = ap_sb[:, :, 512:1024]
            tT_sb = bp_sb[:, :, 0:512]
            gb0_sb = bp_sb[:, :, 512:1024]
            gb1_sb = bp_sb[:, :, 1024:1536]
            ebf_sb = bp_sb[:, :, 1536:1600]
            toute_sb = big.tile([64, 512], F32, tag="toute")
            ocols = big.tile([128, 9], F32, tag="ocols")
            wtile = big.tile([128, 128], F8, tag="warm")

            ap_ap = ap.rearrange("(p t) w -> p t w", p=128)
            tc1_ap = tc1.rearrange("(p t) w -> p t w", p=128)
            bp_ap = bp.rearrange("(p t) w -> p t w", p=128)

            nc.vector.memset(wtile[:], 0.0)
            nc.vector.memset(ocols[:], 0.0)

            # ---- input DMA, priority order matched to consumption order;
            # first piece small so the matmul stream starts ASAP.  The tiny
            # efT load right after piece 1 acts as a spacer so piece 1's
            # packets drain without round-robin interference from piece 2,
            # letting the first matmuls start sooner. ----
            ks = slice(0, 2)
            nc.sync.dma_start(out=ap_sb[:, ks, :], in_=ap_ap[:, ks, :])
            nc.sync.dma_start(out=efT_sb[:], in_=efT[:])
            for ks in (slice(2, 6), slice(6, 10), slice(10, 16)):
                nc.sync.dma_start(out=ap_sb[:, ks, :], in_=ap_ap[:, ks, :])
            for ks in (slice(0, 8), slice(8, 16)):
                nc.sync.dma_start(out=tc1_sb[:, ks, :], in_=tc1_ap[:, ks, :])
            for ks in (slice(0, 4), slice(4, 8), slice(8, 12), slice(12, 16)):
                nc.sync.dma_start(out=bp_sb[:, ks, :], in_=bp_ap[:, ks, :])

            # ---- PE warmup: small FD-128 matmuls to trip HAM un-throttle
            # while the first A-phase data lands; results discarded (the
            # psum tile is reused by the feature term later). ----
            pe_ = pwe_p.tile([64, 512], F32, name="pe")
            for _ in range(NWARM):
                nc.tensor.matmul(pe_[0:1, 0:128], wtile[:, 0:1], wtile[:],
                                 start=True, stop=True, skip_group_check=True)

            # ---- matmul phases: pair-major so consumption tracks DMA
            # arrival; psum tiles cycle through the unified pool ----
            a_tiles = {}

            def mm_phase(lhsT_sb, rhs_sb, n, fuse_a=None, tile_major=False):
                ptiles = {m: pmain_p.tile([128, 512], F32, name="pt")
                          for m in range(4)}
                # tile-major ([3,0,1,2]) in the last phase finishes pair 7
                # early so its STT and the output DMAs overlap the tail MMs
                order = (3, 0, 1, 2) if tile_major else range(4)

                def fuse(m):
                    pair = n * 4 + m
                    if fuse_a is None:
                        ca = asb_p.tile([128, 512], F32, name="ca")
                        nc.scalar.copy(ca[:], ptiles[m][:])
                        a_tiles[pair] = ca
                    else:
                        to = tout_p.tile([128, 512], F32, name="to")
                        nc.vector.scalar_tensor_tensor(
                            out=to[:], in0=a_tiles[pair][:], scalar=1.0,
                            in1=ptiles[m][:], op0=AL.mult, op1=AL.mult,
                            accum_out=ocols[:, pair:pair + 1])

                if tile_major:
                    for m in order:
                        msl = slice(128 * m, 128 * m + 128)
                        for pp in range(8):
                            nc.tensor.matmul(
                                ptiles[m][:], lhsT_sb[:, 2 * pp:2 * pp + 2, msl],
                                rhs_sb[:, 2 * pp:2 * pp + 2, :],
                                start=(pp == 0), stop=(pp == 7),
                                perf_mode=DR, skip_group_check=True)
                        fuse(m)
                        if m == 1:
                            # cols 0:6 + 7:9 are final once pairs 7,4,5 done
                            nc.sync.dma_start(out=out_d[:, 0:6],
                                              in_=ocols[:, 0:6])
                            nc.sync.dma_start(out=out_d[:, 7:9],
                                              in_=ocols[:, 7:9])
                else:
                    for pp in range(8):
                        for m in order:
                            msl = slice(128 * m, 128 * m + 128)
                            nc.tensor.matmul(
                                ptiles[m][:], lhsT_sb[:, 2 * pp:2 * pp + 2, msl],
                                rhs_sb[:, 2 * pp:2 * pp + 2, :],
                                start=(pp == 0), stop=(pp == 7),
                                perf_mode=DR, skip_group_check=True)
                    for m in order:
                        fuse(m)

            mm_phase(csT_sb, tc0_sb, 0)
            mm_phase(csT_sb, tc1_sb, 1)
            mm_phase(tT_sb, gb0_sb, 0, fuse_a=True)

            # ---- feature term: (T Eb)^T [64, 512] for (rblk, dhalf) ----
            for q in range(8):
                nc.tensor.matmul(pe_[:], ebf_sb[:, 2 * q:2 * q + 2, :],
                                 tT_sb[:, 2 * q:2 * q + 2, :],
                                 start=(q == 0), stop=(q == 7),
                                 perf_mode=DR, skip_group_check=True)
            nc.vector.scalar_tensor_tensor(
                out=toute_sb[:], in0=efT_sb[:], scalar=1.0, in1=pe_[:],
                op0=AL.mult, op1=AL.mult, accum_out=ocols[0:64, 8:9])

            mm_phase(tT_sb, gb1_sb, 1, fuse_a=True, tile_major=True)

            nc.sync.dma_start(out=out_d[:, 6:7], in_=ocols[:, 6:7])

    _split_waits(nc)
    return nc


def _prep_inputs(graph, embedding, prob, graph_b, embedding_b, prob_b, tran):
    G = np.asarray(graph, np.float32)
    E = np.asarray(embedding, np.float32)
    GB = np.asarray(graph_b, np.float32)
    EB = np.asarray(embedding_b, np.float32)
    T = np.asarray(tran, np.float32)

    Ts = T * np.float32(TSCALE)
    f8 = lambda x: np.ascontiguousarray(x).astype(FP8)
    in_maps = []
    for idx in range(NCORES):
        r, c = idx // 2, idx % 2
        rblk = slice(512 * r, 512 * r + 512)
        dh = slice(64 * c, 64 * c + 64)
        c0 = slice(1024 * c, 1024 * c + 512)
        c1 = slice(1024 * c + 512, 1024 * c + 1024)
        ap = np.empty((N, 1024), FP8)
        ap[:, 0:512] = G[rblk, :].T.astype(FP8)
        ap[:, 512:1024] = Ts[:, c0].astype(FP8)
        bp = np.empty((N, 1600), FP8)
        bp[:, 0:512] = Ts[rblk, :].T.astype(FP8)
        bp[:, 512:1024] = GB[:, c0].astype(FP8)
        bp[:, 1024:1536] = GB[:, c1].astype(FP8)
        bp[:, 1536:1600] = EB[:, dh].astype(FP8)
        in_maps.append({
            "ap": ap,
            "tc1": f8(Ts[:, c1]),
            "bp": bp,
            "efT": np.ascontiguousarray(E[rblk, dh].T, dtype=np.float32),
        })
    return in_maps


def _reduce(results, graph, embedding, prob, graph_b, embedding_b, prob_b, tran):
    S_main = 0.0
    S_emb = 0.0
    for idx in range(NCORES):
        o = np.asarray(results[idx]["out"], np.float64)
        S_main += o[:, 0:8].sum()
        S_emb += o[0:64, 8].sum()
    S_main /= TSCALE * TSCALE
    S_emb /= TSCALE

    # rank-1 corrections on host in f64 (tiny O(N^2) work)
    G = np.asarray(graph, np.float64)
    GB = np.asarray(graph_b, np.float64)
    T = np.asarray(tran, np.float64)
    E = np.asarray(embedding, np.float64)
    EB = np.asarray(embedding_b, np.float64)
    ps = np.asarray(prob, np.float64).reshape(N)
    pt = np.asarray(prob_b, np.float64).reshape(N)

    f1 = (G * G) @ ps
    f2 = (GB * GB) @ pt
    r = T.sum(axis=1)
    c = T.sum(axis=0)
    nE = (E * E).sum(axis=1)
    nEB = (EB * EB).sum(axis=1)

    total = (((f1 + 0.5 * nE) * r).sum()
             + ((f2 + 0.5 * nEB) * c).sum()
             - 2.0 * S_main
             - S_emb)
    return np.float32(total)


def run_spmd(in_maps, trace=False, **kw):
    if "nc" not in _cache:
        _cache["nc"] = _build()
    return bass_utils.run_bass_kernel_spmd(
        _cache["nc"], in_maps, list(range(NCORES)), trace=trace, **kw)


def kernel(graph, embedding, prob, graph_b, embedding_b, prob_b, tran,
           weights, ole_coeff, idx):
    in_maps = _prep_inputs(graph, embedding, prob, graph_b, embedding_b,
                           prob_b, tran)
    last_err = None
    for _attempt in range(3):
        try:
            res = run_spmd(in_maps)
            return _reduce(res.results, graph, embedding, prob,
                           graph_b, embedding_b, prob_b, tran)
        except Exception as e:  # transient NRT device errors seen under axon
            last_err = e
    raise last_err



# revision 2
# speedup vs baseline: 1.0226x; 1.0226x over previous
"""Trainium2 Bass kernel for the FGWF objective (v2):

    out = sum(cost_mat(graph, graph_b, prob, prob_b, tran, embedding, embedding_b) * tran)

Decomposition:
  sum(cost*T) = sum_i (f1_i + 0.5*||E_i||^2) * rowsum(T)_i
              + sum_j (f2_j + 0.5*||Eb_j||^2) * colsum(T)_j
              - 2 * <G @ T, T @ Gb>              (two 2048^3 matmuls, fp8 DoubleRow)
              - <E, T @ Eb>                      (feature cross term, host BLAS)
  with f1 = (G^2) @ p_s, f2 = (Gb^2) @ p_t.

Device computes ONLY the O(N^3) term <G@T, T@Gb>.  All O(N^2) and O(N^2 D)
corrections run on the host (f64 / f32 BLAS) during the reduction.

Sharding over 8 NeuronCores: 2D grid (4 row-blocks x 2 col-blocks).  Core
(r, c) computes A = G[rblk] @ T[:, chalf] and B = T[rblk] @ Gb[:, chalf] in
two column-half rounds.  Within a round the 4 A psum tiles and 4 B psum
tiles (8 PSUM banks) are produced back-to-back by the PE, then fused by a
DVE scalar_tensor_tensor reading both operands straight from PSUM
(accumulating per-partition dots into ocols).  No PSUM->SBUF evacuation.
tran is pre-scaled by 2^20 so fp8-e4m3 avoids subnormal flush.
"""
import numpy as np
import ml_dtypes

import concourse.bass as bass
from concourse import mybir
from concourse import bass_utils
from concourse.tile import TileContext

FP8 = ml_dtypes.float8_e4m3
F32 = mybir.dt.float32
F8 = mybir.dt.float8e4
BF16 = mybir.dt.bfloat16
TSCALE = 2.0 ** 20
N = 2048
NCORES = 8
NWARM = 25

_cache = {}


def _split_waits(nc):
    """The walrus build here rejects >1 sem-wait per instruction; hoist extra
    waits onto preceding same-engine nops."""
    MAXW = 1
    for fn in nc.m.functions:
        for b in fn.blocks:
            out = []
            changed = False
            for inst in b.instructions:
                si = inst.sync_info
                waits = list(si.on_wait) if si and si.on_wait else []
                if len(waits) > MAXW:
                    changed = True
                    k = 0
                    while len(waits) > MAXW:
                        chunk, waits = waits[:MAXW], waits[MAXW:]
                        out.append(mybir.InstNoOp(
                            name=f"{inst.name}-wsplit{k}", engine=inst.engine,
                            sync_info=mybir.SyncInfo(on_wait=chunk, on_update=[]),
                            bass_nofuse=True))
                        k += 1
                    inst.sync_info = mybir.SyncInfo(
                        on_wait=waits,
                        on_update=list(si.on_update) if si.on_update else [])
                out.append(inst)
            if changed:
                b.instructions = out


def _build():
    nc = bass.Bass()
    AL = mybir.AluOpType
    DR = mybir.MatmulPerfMode.DoubleRow

    # per-core inputs (host-sharded, consumption-ordered pieces).
    # ap rows pack {G[rblk]^T | Ts[:,c0]} so one chunk-pair DMA feeds both
    # operands of an A-phase pp-step; bp rows pack {Ts[rblk]^T | Gb[:,c0]}.
    ap = nc.declare_dram_parameter("ap", [N, 1024], F8, isOutput=False)
    bp = nc.declare_dram_parameter("bp", [N, 1024], F8, isOutput=False)
    tc1 = nc.declare_dram_parameter("tc1", [N, 512], F8, isOutput=False)
    gb1 = nc.declare_dram_parameter("gb1", [N, 512], F8, isOutput=False)

    out_d = nc.declare_dram_parameter("out", [128, 8], F32, isOutput=True)

    with TileContext(nc) as tc:
        with (
            tc.tile_pool(name="big", bufs=1) as big,
            tc.tile_pool(name="tout", bufs=2) as tout_p,
            tc.tile_pool(name="evac", bufs=8) as evac_p,
            tc.tile_pool(name="pmain", bufs=1, space="PSUM") as pmain_p,
        ):
            ap_sb = big.tile([128, 16, 1024], F8, tag="ap")
            bp_sb = big.tile([128, 16, 1024], F8, tag="bp")
            tc1_sb = big.tile([128, 16, 512], F8, tag="tc1")
            gb1_sb = big.tile([128, 16, 512], F8, tag="gb1")
            ocols = big.tile([128, 8], F32, tag="ocols")
            wtile = big.tile([128, 128], F8, tag="warm")

            csT_sb = ap_sb[:, :, 0:512]
            tc0_sb = ap_sb[:, :, 512:1024]
            tT_sb = bp_sb[:, :, 0:512]
            gb0_sb = bp_sb[:, :, 512:1024]

            ap_ap = ap.rearrange("(p t) w -> p t w", p=128)
            bp_ap = bp.rearrange("(p t) w -> p t w", p=128)
            tc1_ap = tc1.rearrange("(p t) w -> p t w", p=128)
            gb1_ap = gb1.rearrange("(p t) w -> p t w", p=128)

            nc.vector.memset(wtile[:], 0.0)
            nc.vector.memset(ocols[:], 0.0)

            # ---- input DMA in consumption order; first piece small so the
            # matmul stream starts ASAP.  One HWDGE ring => FIFO arrival. ----
            for k in range(8):
                ks = slice(2 * k, 2 * k + 2)
                nc.sync.dma_start(out=ap_sb[:, ks, :], in_=ap_ap[:, ks, :])
            for k in range(4):
                ks = slice(4 * k, 4 * k + 4)
                nc.sync.dma_start(out=bp_sb[:, ks, :], in_=bp_ap[:, ks, :])
            for k in range(4):
                ks = slice(4 * k, 4 * k + 4)
                nc.sync.dma_start(out=tc1_sb[:, ks, :], in_=tc1_ap[:, ks, :])
            for ks in (slice(0, 8), slice(8, 16)):
                nc.sync.dma_start(out=gb1_sb[:, ks, :], in_=gb1_ap[:, ks, :])

            at = [pmain_p.tile([128, 512], F32, name=f"pa{m}") for m in range(4)]
            bt = [pmain_p.tile([128, 512], F32, name=f"pb{m}") for m in range(4)]

            # ---- PE warmup while the first DMA piece lands ----
            for _ in range(NWARM):
                nc.tensor.matmul(at[0][0:1, 0:128], wtile[:, 0:1], wtile[:],
                                 start=True, stop=True, skip_group_check=True)

            def mm_group(lhsT_sb, rhs_sb, ptiles, tile_major=False):
                if tile_major:
                    for m in range(4):
                        msl = slice(128 * m, 128 * m + 128)
                        for pp in range(8):
                            nc.tensor.matmul(
                                ptiles[m][:], lhsT_sb[:, 2 * pp:2 * pp + 2, msl],
                                rhs_sb[:, 2 * pp:2 * pp + 2, :],
                                start=(pp == 0), stop=(pp == 7),
                                perf_mode=DR, skip_group_check=True)
                else:
                    for pp in range(8):
                        for m in range(4):
                            msl = slice(128 * m, 128 * m + 128)
                            nc.tensor.matmul(
                                ptiles[m][:], lhsT_sb[:, 2 * pp:2 * pp + 2, msl],
                                rhs_sb[:, 2 * pp:2 * pp + 2, :],
                                start=(pp == 0), stop=(pp == 7),
                                perf_mode=DR, skip_group_check=True)

            def evac(m):
                ae = evac_p.tile([128, 512], BF16, name="ae")
                nc.scalar.copy(ae[:], at[m][:])
                return ae

            def fuse(ae, m, col):
                to = tout_p.tile([128, 512], F32, name="to")
                nc.vector.scalar_tensor_tensor(
                    out=to[:], in0=ae[:], scalar=1.0, in1=bt[m][:],
                    op0=AL.mult, op1=AL.mult,
                    accum_out=ocols[:, col:col + 1])

            # ---- half 0: A group, evacuate A psum to SBUF bf16 (scalar
            # engine, overlaps B group), B group, fuse ----
            mm_group(csT_sb, tc0_sb, at)
            aes = [evac(m) for m in range(4)]
            mm_group(tT_sb, gb0_sb, bt)
            for m in range(4):
                fuse(aes[m], m, m)

            # ---- half 1: psum tiles reused; final B tile-major so fusions
            # and the output DMA overlap the tail matmuls ----
            mm_group(csT_sb, tc1_sb, at)
            aes = [evac(m) for m in range(4)]
            mm_group(tT_sb, gb1_sb, bt, tile_major=True)
            for m in range(4):
                fuse(aes[m], m, 4 + m)
                if m == 2:
                    nc.sync.dma_start(out=out_d[:, 0:7], in_=ocols[:, 0:7])
            nc.sync.dma_start(out=out_d[:, 7:8], in_=ocols[:, 7:8])

    _split_waits(nc)
    return nc


def _prep_inputs(graph, embedding, prob, graph_b, embedding_b, prob_b, tran):
    G = np.asarray(graph, np.float32)
    GB = np.asarray(graph_b, np.float32)
    T = np.asarray(tran, np.float32)

    Ts = T * np.float32(TSCALE)
    f8 = lambda x: np.ascontiguousarray(x).astype(FP8)
    in_maps = []
    for idx in range(NCORES):
        r, c = idx // 2, idx % 2
        rblk = slice(512 * r, 512 * r + 512)
        c0 = slice(1024 * c, 1024 * c + 512)
        c1 = slice(1024 * c + 512, 1024 * c + 1024)
        ap = np.empty((N, 1024), FP8)
        ap[:, 0:512] = G[rblk, :].T.astype(FP8)
        ap[:, 512:1024] = Ts[:, c0].astype(FP8)
        bp = np.empty((N, 1024), FP8)
        bp[:, 0:512] = Ts[rblk, :].T.astype(FP8)
        bp[:, 512:1024] = GB[:, c0].astype(FP8)
        in_maps.append({
            "ap": ap,
            "bp": bp,
            "tc1": f8(Ts[:, c1]),
            "gb1": f8(GB[:, c1]),
        })
    return in_maps


def _reduce(results, graph, embedding, prob, graph_b, embedding_b, prob_b, tran):
    S_main = 0.0
    for idx in range(NCORES):
        o = np.asarray(results[idx]["out"], np.float64)
        S_main += o.sum()
    S_main /= TSCALE * TSCALE

    # rank-1 corrections + feature term on host (tiny O(N^2)/O(N^2 D) work)
    G = np.asarray(graph, np.float64)
    GB = np.asarray(graph_b, np.float64)
    T = np.asarray(tran, np.float64)
    E = np.asarray(embedding, np.float64)
    EB = np.asarray(embedding_b, np.float64)
    ps = np.asarray(prob, np.float64).reshape(N)
    pt = np.asarray(prob_b, np.float64).reshape(N)

    f1 = (G * G) @ ps
    f2 = (GB * GB) @ pt
    r = T.sum(axis=1)
    c = T.sum(axis=0)
    nE = (E * E).sum(axis=1)
    nEB = (EB * EB).sum(axis=1)

    T32 = np.asarray(tran, np.float32)
    S_emb = float(np.einsum(
        "ij,ij->", np.asarray(embedding, np.float32),
        T32 @ np.asarray(embedding_b, np.float32), dtype=np.float64))

    total = (((f1 + 0.5 * nE) * r).sum()
             + ((f2 + 0.5 * nEB) * c).sum()
             - 2.0 * S_main
             - S_emb)
    return np.float32(total)


def run_spmd(in_maps, trace=False, **kw):
    if "nc" not in _cache:
        _cache["nc"] = _build()
    return bass_utils.run_bass_kernel_spmd(
        _cache["nc"], in_maps, list(range(NCORES)), trace=trace, **kw)


def kernel(graph, embedding, prob, graph_b, embedding_b, prob_b, tran,
           weights, ole_coeff, idx):
    in_maps = _prep_inputs(graph, embedding, prob, graph_b, embedding_b,
                           prob_b, tran)
    last_err = None
    for _attempt in range(3):
        try:
            res = run_spmd(in_maps)
            return _reduce(res.results, graph, embedding, prob,
                           graph_b, embedding_b, prob_b, tran)
        except Exception as e:  # transient NRT device errors seen under axon
            last_err = e
    raise last_err
